# revision 11
# baseline (speedup 1.0000x reference)
"""Multi-head causal attention (B=2, S=2048, D=1024, H=16) on 8 Trainium2
NeuronCores.

Sharding: tensor-parallel over heads - 2 heads per core. Each core computes
its heads' Q/K/V projections, causal attention, and a partial output
projection (row-parallel over the head dims); the host sums the 8 partials
and adds the output bias.

v2 (bf16): all matmul operands and DMA'd activations are bfloat16
(accumulation stays fp32 in PSUM). vs the f32r baseline this halves HBM
traffic and SBUF footprint and doubles DVE throughput on SBUF-only ops.

Device layout is fully "transposed" (features on partitions, tokens on the
free axis):
  - QKV projection:  QKVT[f, t]  via lhsT=W^T tiles, rhs=X^T tiles
  - V is PE-transposed to token-major and packed (with a ones column) into
    `vaug`; Q^T/K^T stay feature-major
  - scores^T[k, q] = KT_tile^T @ QT_block  for both heads of a (b,qb,kt)
    into one 2-bank PSUM pair, row-packed via tile_position (h*64, 0) so
    the two matmuls run concurrently on the PE's row groups
  - ONE exp per (b,qb,kt) over a 2-region AP covering both heads' valid
    columns; causal masking only touches the 128-wide diagonal band, done
    in-place on GPSIMD via affine_select (DVE untouched)
  - attn^T[hd, q] accumulates over k tiles in PSUM; the ones column of
    vaug gives the softmax denominators in row 64
  - normalization: reciprocal_approx_fast + partition-broadcast multiply
  - out^T[e, t] partial = WoutT^T @ attnT, summed across cores on the host
"""

import contextlib

import numpy as np

import bass_rust
import concourse.bass as bass
import concourse.mybir as mybir
from concourse.bass_utils import run_bass_kernel_spmd
from concourse.tile import TileContext
from concourse.masks import make_identity

F32 = mybir.dt.float32
F32R = mybir.dt.float32r
BF16 = mybir.dt.bfloat16

B, S, D, H = 2, 2048, 1024, 16
HD = D // H            # 64
NCORES = 8
HPC = H // NCORES      # heads per core = 2
DSL = HPC * HD         # feature slice per core = 128
T = B * S              # 4096 tokens
NT = T // 512          # 8 token blocks of 512
ND = D // 128          # 8 d-tiles
NKT = S // 128         # 16 k-tiles per batch
NQB = S // 512         # 4 q-blocks per batch
NSB = NT // 2          # 4 super-blocks of 1024 tokens


def _split_multi_waits(nc):
    """This walrus build accepts only ONE sync-wait per instruction. Hoist
    all-but-one wait of any multi-wait instruction onto same-engine NoOps
    placed immediately before it."""
    n = 0
    for f in nc.m.functions:
        for blk in f.blocks:
            il = blk.instructions
            new = []
            changed = False
            for inst in il:
                si = inst.sync_info
                waits = list(si.on_wait) if si is not None and si.on_wait else []
                if len(waits) > 1:
                    changed = True
                    for w in waits[:-1]:
                        nop = mybir.InstNoOp(
                            name=f"I-waitsplit-{nc.next_id()}", ins=[], outs=[]
                        )
                        nop.engine = inst.engine
                        nop.sync_info = bass_rust.SyncInfo(on_wait=[w], on_update=[])
                        new.append(nop)
                        n += 1
                    inst.sync_info = bass_rust.SyncInfo(
                        on_wait=[waits[-1]], on_update=list(si.on_update or [])
                    )
                new.append(inst)
            if changed:
                blk.instructions = new
    return n


def _build(loop_n=None, loop_phase=None, dbg=False):
    nc = bass.Bass("TRN2", target_bir_lowering=False, debug=False)

    XT = nc.declare_dram_parameter("XT", [D, T], BF16, isOutput=False)
    WQKVT = nc.declare_dram_parameter("WQKVT", [D, 3 * DSL], BF16, isOutput=False)
    BQKV = nc.declare_dram_parameter("BQKV", [3 * DSL, 1], F32, isOutput=False)
    WOUTT = nc.declare_dram_parameter("WOUTT", [DSL, D], BF16, isOutput=False)
    OUTT = nc.declare_dram_parameter("OUTT", [D, T], BF16, isOutput=True)
    if dbg:
        QDBG = nc.declare_dram_parameter("QDBG", [128, T], BF16, isOutput=True)
        KDBG = nc.declare_dram_parameter("KDBG", [128, T], BF16, isOutput=True)
        VDBG = nc.declare_dram_parameter(
            "VDBG", [128, (T // 128) * 2 * (HD + 1)], BF16, isOutput=True)
        ADBG = nc.declare_dram_parameter("ADBG", [128, T], BF16, isOutput=True)

    EXP = mybir.ActivationFunctionType.Exp
    scale = 1.0 / np.sqrt(HD)

    with TileContext(nc) as tc:
        with (
            tc.tile_pool(name="const", bufs=1) as const,
            tc.tile_pool(name="big", bufs=1) as big,
            tc.tile_pool(name="xt", bufs=12) as xtp,
            tc.tile_pool(name="vsb", bufs=3) as vsbp,
            tc.tile_pool(name="ep", bufs=8) as ep,
            tc.tile_pool(name="work", bufs=8) as work,
            tc.tile_pool(name="obp", bufs=6) as obp,
            tc.tile_pool(name="accps", bufs=2, space="PSUM") as accps,
            tc.tile_pool(name="s2ps", bufs=2, space="PSUM") as s2ps,
            tc.tile_pool(name="strps", bufs=2, space="PSUM") as strps,
            contextlib.ExitStack() as _loop_ctx,
        ):
            def phase_loop(p):
                if loop_n is not None and (loop_phase is None or loop_phase == p):
                    return tc.For_i(0, loop_n, 1)
                return contextlib.nullcontext()

            # ---- constants / weights ------------------------------------
            wq = []
            for d in range(ND):
                w = const.tile([128, 3 * DSL], BF16, name=f"wq{d}")
                nc.sync.dma_start(out=w, in_=WQKVT[d * 128:(d + 1) * 128, :])
                wq.append(w)
            woutt = const.tile([DSL, D], BF16, name="woutt")
            nc.sync.dma_start(out=woutt, in_=WOUTT[:, :])
            bias = []
            for f in range(3):
                bf = const.tile([DSL, 1], F32, name=f"bias{f}")
                nc.sync.dma_start(out=bf, in_=BQKV[f * DSL:(f + 1) * DSL, :])
                bias.append(bf)
            ident_f = const.tile([128, 128], F32, name="ident_f")
            make_identity(nc, ident_f)
            ident = const.tile([128, 128], BF16, name="ident")
            with nc.allow_low_precision(reason="identity is exact in bf16"):
                nc.vector.tensor_copy(ident, ident_f)
            ones64r_f = const.tile([1, HD], F32, name="ones64r_f")
            nc.vector.memset(ones64r_f, 1.0)
            onesr = const.tile([1, HD], BF16, name="onesr")
            with nc.allow_low_precision(reason="ones are exact in bf16"):
                nc.vector.tensor_copy(onesr, ones64r_f)

            # ---- persistent activations ---------------------------------
            # Q^T, K^T feature-major (two heads stacked on partitions)
            qkvt = [big.tile([128, T], BF16, name=f"qkvt{f}") for f in range(2)]
            # V token-major + ones column, per (k-tile, head)
            nones = (T // 128) * 2
            vaug = big.tile([128, nones * (HD + 1)], BF16, name="vaug")
            attnt = big.tile([128, T], BF16, name="attnt")
            ones64_f = const.tile([128, 64], F32, name="ones64_f")
            nc.vector.memset(ones64_f, 1.0)
            with nc.allow_low_precision(reason="ones are exact in bf16"):
                nc.vector.tensor_copy(vaug[:, HD::HD + 1], ones64_f[:, 0:nones])

            # ---- work-item emitters -------------------------------------
            def emit_xt_dmas(sb2):
                xt = []
                for d in range(ND):
                    x = xtp.tile([128, 1024], BF16, name="xtc", tag="xt")
                    nc.sync.dma_start(
                        out=x,
                        in_=XT[d * 128:(d + 1) * 128,
                               sb2 * 1024:(sb2 + 1) * 1024],
                    )
                    xt.append(x)
                return xt

            def emit_qkv_group(xt, sb2, th, f):
                """One [128, 512] projection tile: 8 matmuls + bias add.
                For V (f == 2) also transpose to token-major into vaug."""
                t = sb2 * 2 + th
                ps = strps.tile([128, 512], F32, name="ps_qkv", tag="s")
                for d in range(ND):
                    nc.tensor.matmul(
                        ps,
                        wq[d][:, f * DSL:(f + 1) * DSL],
                        xt[d][:, th * 512:(th + 1) * 512],
                        start=(d == 0),
                        stop=(d == ND - 1),
                    )
                with nc.allow_low_precision(reason="bf16 activations"):
                    if f < 2:
                        nc.vector.tensor_scalar_add(
                            qkvt[f][:, t * 512:(t + 1) * 512], ps, bias[f]
                        )
                        return
                    vsb = vsbp.tile([128, 512], BF16, name="vsb", tag="v")
                    nc.vector.tensor_scalar_add(vsb, ps, bias[2])
                    tp = strps.tile([128, 512], BF16, name="ps_t", tag="s")
                    for i in range(4):
                        nc.tensor.transpose(
                            tp[:, i * 128:(i + 1) * 128],
                            vsb[:, i * 128:(i + 1) * 128], ident
                        )
                    # one strided copy: [kk, (i, h, hd)] -> vaug columns
                    dst = (
                        vaug[:, t * 8 * (HD + 1):(t + 1) * 8 * (HD + 1)]
                        .rearrange("p (i x) -> p i x", i=4)[:, :, 0:2 * (HD + 1)]
                        .rearrange("p i (h y) -> p i h y", h=2)[:, :, :, 0:HD]
                    )
                    src = tp.rearrange("p (i h y) -> p i h y", i=4, h=2)
                    nc.vector.tensor_copy(dst, src)

            def emit_scores(b, qb, kt):
                """Both heads' scores^T into one 2-bank PSUM pair + one exp.
                Exact causal slicing: diagonal tile j covers q columns
                [128j, 512); the 128-wide band is masked in place on GPSIMD."""
                qc = b * S + qb * 512
                ktg = b * NKT + kt
                diag = kt >= 4 * qb
                off = 128 * (kt - 4 * qb) if diag else 0
                off = 0  # DIAG: full width while debugging
                pair = s2ps.tile([128, 1024], F32, name="ps_s2", tag="s2")
                for h in range(2):
                    nc.tensor.matmul(
                        pair[:, h * 512 + off:(h + 1) * 512],
                        qkvt[1][h * HD:(h + 1) * HD, ktg * 128:(ktg + 1) * 128],
                        qkvt[0][h * HD:(h + 1) * HD, qc + off:qc + 512],
                        start=True, stop=True, tile_position=(h * HD, 0),
                    )
                expt = ep.tile([128, 1024], BF16, name="expt", tag="e")
                with nc.allow_low_precision(reason="softmax probs in bf16"):
                    for h in range(2):  # DIAG: one exp per bank
                        nc.scalar.activation(
                            expt[:, h * 512 + off:(h + 1) * 512],
                            pair[:, h * 512 + off:(h + 1) * 512],
                            EXP, scale=scale)
                if diag:
                    j = kt - 4 * qb
                    for h in range(2):
                        band = expt[:, h * 512 + off:(h + 1) * 512]
                        nc.gpsimd.affine_select(
                            out=band, in_=band,
                            compare_op=mybir.AluOpType.is_ge,
                            fill=0.0, base=-(128 * j - off),
                            channel_multiplier=-1,
                            pattern=[[1, 512 - off]],
                        )
                return expt, off

            def emit_attnv(ps_o, b, qb, kt, seq, h, src_off, nkt):
                expt, off = src_off
                ktg = b * NKT + kt
                va = vaug[:, (ktg * 2 + h) * (HD + 1):
                          (ktg * 2 + h + 1) * (HD + 1)]
                nc.tensor.matmul(
                    ps_o[h][:, off:512], va,
                    expt[:, h * 512 + off:(h + 1) * 512],
                    start=(seq == 0), stop=(seq == nkt - 1),
                )

            def make_epilogue(ps_o, b, qb):
                def epi():
                    qc = b * S + qb * 512
                    for h in range(2):
                        recipb = work.tile([1, 512], BF16, name="recipb",
                                           tag="rb")
                        with nc.allow_low_precision(reason="softmax denom"):
                            nc.vector.reciprocal(recipb, ps_o[h][HD:HD + 1, :])
                        ps_b = strps.tile([HD, 512], F32, name="ps_b", tag="s")
                        nc.tensor.matmul(ps_b, onesr, recipb,
                                         start=True, stop=True)
                        bc = work.tile([HD, 512], BF16, name="bc", tag="bc")
                        with nc.allow_low_precision(reason="softmax normalize"):
                            nc.scalar.copy(bc, ps_b)
                            nc.vector.tensor_mul(
                                attnt[h * HD:(h + 1) * HD, qc:qc + 512],
                                ps_o[h][0:HD, :], bc,
                            )
                return epi

            in_tail = [False]  # True once all exps are emitted (flush)

            def make_outproj(b, qb):
                # this outproj gets consumed during the NEXT q-block; route
                # evacuations to ACT only when that block is exp-light
                act_evac = qb in (3, 0)

                def opj():
                    tb = b * S + qb * 512
                    for e in range(ND):
                        ps = strps.tile([128, 512], F32, name="ps_out", tag="s")
                        nc.tensor.matmul(
                            ps,
                            woutt[:, e * 128:(e + 1) * 128],
                            attnt[:, tb:tb + 512],
                            start=True, stop=True,
                        )
                        ob = obp.tile([128, 512], BF16, name="ob", tag="ob")
                        with nc.allow_low_precision(reason="bf16 partials"):
                            if (act_evac or in_tail[0]) and e % 2 == 1:
                                nc.scalar.copy(ob, ps)
                            else:
                                nc.vector.tensor_copy(ob, ps)
                        nc.sync.dma_start(
                            out=OUTT[e * 128:(e + 1) * 128, tb:tb + 512],
                            in_=ob,
                        )
                return opj

            # ---- interleaved emission -----------------------------------
            with phase_loop(0):
                qkv_items = []  # deferred (sb2 >= 2) qkv groups

                def run_qkv_sb(sb2, defer):
                    xt = emit_xt_dmas(sb2)
                    for th in range(2):
                        for f in range(3):
                            if defer:
                                qkv_items.append(
                                    (lambda xt=xt, sb2=sb2, th=th, f=f:
                                     emit_qkv_group(xt, sb2, th, f))
                                )
                            else:
                                emit_qkv_group(xt, sb2, th, f)

                run_qkv_sb(0, defer=False)
                run_qkv_sb(1, defer=False)
                run_qkv_sb(2, defer=True)
                run_qkv_sb(3, defer=True)

                pending = []

                def pop_pending():
                    # epilogue/outproj first: releases accumulator banks
                    if pending:
                        pending.pop(0)()
                    elif qkv_items:
                        qkv_items.pop(0)()

                def pop_qkv():
                    # no pending fallback: draining future epilogue/outproj
                    # early starves the q-block boundaries and the tail
                    if qkv_items:
                        qkv_items.pop(0)()

                for b in range(B):
                    for qb in range(NQB):
                        nkt = 4 * qb + 4
                        # diagonal k-tiles first: their GPSIMD band masks
                        # happen early, so the end of the q-block is pure
                        # exp->matmul with no Pool dependency; the first
                        # emitted tile (kt=4*qb, j=0) has off=0 so the
                        # PSUM clear covers the full accumulator width.
                        kt_order = list(range(4 * qb, nkt)) + list(range(4 * qb))
                        ps_o = [
                            accps.tile([HD + 1, 512], F32,
                                       name=f"ps_o{h}", tag="o")
                            for h in range(2)
                        ]
                        # lookahead: scores/exp run DEPTH k-tiles ahead of
                        # the attn@V consuming them, covering exp+mask
                        # latency with matmul groups.
                        DEPTH = 2
                        window = []
                        for i in range(min(DEPTH, nkt)):
                            window.append((i, emit_scores(b, qb, kt_order[i])))
                            if i == 0:
                                pop_pending()
                        for i in range(DEPTH, nkt):
                            cur = (i, emit_scores(b, qb, kt_order[i]))
                            if i == nkt - 2:
                                pop_pending()
                            elif i % 2 == 1:
                                pop_qkv()
                            seq, old = window.pop(0)
                            for h in range(2):
                                emit_attnv(ps_o, b, qb, kt_order[seq], seq, h,
                                           old, nkt)
                            window.append(cur)
                        for seq, srcs in window:
                            for h in range(2):
                                emit_attnv(ps_o, b, qb, kt_order[seq], seq, h,
                                           srcs, nkt)
                        pending.append(make_epilogue(ps_o, b, qb))
                        pending.append(make_outproj(b, qb))
                for fn in qkv_items:
                    fn()
                in_tail[0] = True
                for fn in pending:
                    fn()
                if dbg:
                    nc.sync.dma_start(out=QDBG[:, :], in_=qkvt[0])
                    nc.sync.dma_start(out=KDBG[:, :], in_=qkvt[1])
                    nc.sync.dma_start(out=VDBG[:, :], in_=vaug)
                    nc.sync.dma_start(out=ADBG[:, :], in_=attnt)

    nc.finalize()
    _split_multi_waits(nc)
    return nc


_NC = None
LAST_EXEC_TIME_NS = None


def _bf16(a):
    import ml_dtypes
    return np.ascontiguousarray(np.asarray(a, dtype=np.float32)).astype(
        ml_dtypes.bfloat16
    )


def make_in_maps(X, W_qkv, b_qkv, W_out, b_out):
    X = np.asarray(X, dtype=np.float32)
    W_qkv = np.asarray(W_qkv, dtype=np.float32)
    b_qkv = np.asarray(b_qkv, dtype=np.float32)
    W_out = np.asarray(W_out, dtype=np.float32)

    XTv = _bf16(X.reshape(T, D).T)

    in_maps = []
    for c in range(NCORES):
        sl = slice(c * DSL, (c + 1) * DSL)
        wc = np.concatenate(
            [W_qkv[c * DSL:(c + 1) * DSL],
             W_qkv[D + c * DSL:D + (c + 1) * DSL],
             W_qkv[2 * D + c * DSL:2 * D + (c + 1) * DSL]],
            axis=0,
        )
        wqkvt = _bf16(wc.T)
        bq = np.concatenate(
            [b_qkv[sl], b_qkv[D + sl.start:D + sl.stop],
             b_qkv[2 * D + sl.start:2 * D + sl.stop]]
        ).reshape(3 * DSL, 1)
        woutt = _bf16(W_out[:, sl].T)
        in_maps.append(
            {
                "XT": XTv,
                "WQKVT": wqkvt,
                "BQKV": np.ascontiguousarray(bq.astype(np.float32)),
                "WOUTT": woutt,
            }
        )
    return in_maps


def kernel(X, W_qkv, b_qkv, W_out, b_out):
    global _NC, LAST_EXEC_TIME_NS
    b_out = np.asarray(b_out, dtype=np.float32)
    in_maps = make_in_maps(X, W_qkv, b_qkv, W_out, b_out)

    if _NC is None:
        _NC = _build()
    res = run_bass_kernel_spmd(_NC, in_maps, core_ids=list(range(NCORES)))
    LAST_EXEC_TIME_NS = res.exec_time_ns

    total = res.results[0]["OUTT"].astype(np.float32)
    for r in res.results[1:]:
        total += r["OUTT"].astype(np.float32)
    out = total.T + b_out
    return np.ascontiguousarray(out.reshape(B, S, D).astype(np.float32))
